# revision 4
# baseline (speedup 1.0000x reference)
"""DeepSeekV3 MoE router on 8 TRN2 NeuronCores (Bass/Tile).

Strategy (hardcoded for T=8192, D=7168, E=256, top-k=8, 8 groups, top-4 groups):
  - Data-parallel over tokens: each of 8 cores handles 1024 tokens.
  - Router weight kernel_DE and bias replicated to every core.
  - Host pre-arranges x into the lhsT chunk layout the PE needs
    (contraction dim D on partitions), so no on-chip transposes.
  - Per 128-token tile: 56 accumulating fp32 matmuls -> PSUM scores,
    sigmoid on ACT, grouped top-2 / top-4-groups / top-8 with DVE
    Max8 / max_index / match_replace ops, normalize, DMA out.
"""

import sys

for p in ("/opt/trn_rl_repo", "/root/.axon_site/_ro/trn_rl_repo"):
    if p not in sys.path:
        sys.path.insert(0, p)

import numpy as np

T = 8192
D = 7168
E = 256
TOP_K = 8
N_GROUPS = 8
EPG = E // N_GROUPS  # experts per group = 32
TOPK_GROUPS = 4
SCALE = 2.5
N_CORES = 8
TPC = T // N_CORES  # tokens per core = 1024
N_TILES = TPC // 128  # 8 token tiles per core
KC = D // 128  # 56 contraction chunks

_CACHE = {}


def _build_nc():
    import concourse.bacc as bacc
    import concourse.mybir as mybir
    import concourse.tile as tile

    f32 = mybir.dt.float32
    u32 = mybir.dt.uint32
    X = mybir.AxisListType.X
    Alu = mybir.AluOpType

    nc = bacc.Bacc(trn_type="TRN2")
    x_d = nc.declare_dram_parameter("x", [128, N_TILES, KC, 128], f32, isOutput=False)
    w_d = nc.declare_dram_parameter("w", [128, KC, E], f32, isOutput=False)
    b_d = nc.declare_dram_parameter("bias", [128, E], f32, isOutput=False)
    ow_d = nc.declare_dram_parameter("out_w", [N_TILES, 128, TOP_K], f32, isOutput=True)
    oi_d = nc.declare_dram_parameter("out_idx", [N_TILES, 128, TOP_K], u32, isOutput=True)

    with tile.TileContext(nc) as tc:
        with (
            tc.tile_pool(name="const", bufs=1) as cpool,
            tc.tile_pool(name="xin", bufs=3) as xpool,
            tc.tile_pool(name="work", bufs=2) as wpool,
            tc.tile_pool(name="small", bufs=2) as spool,
            tc.tile_pool(name="psum", bufs=2, space="PSUM") as ppool,
        ):
            w_sb = cpool.tile([128, KC, E], f32)
            nc.sync.dma_start(w_sb[:], w_d[:])
            bias_sb = cpool.tile([128, E], f32)
            nc.sync.dma_start(bias_sb[:], b_d[:])

            for tt in range(N_TILES):
                x_sb = xpool.tile([128, KC, 128], f32, tag="x")
                nc.sync.dma_start(x_sb[:], x_d[:, tt])

                ps = ppool.tile([128, E], f32, tag="ps")
                for k in range(KC):
                    nc.tensor.matmul(
                        ps[:],
                        lhsT=x_sb[:, k],
                        rhs=w_sb[:, k],
                        start=(k == 0),
                        stop=(k == KC - 1),
                    )

                # g = sigmoid(scores) = 1/(1+exp(-z)), decomposed exactly as
                # XLA lowers logistic on this backend (bitwise-matching the
                # reference selection): ACT Exp(scale=-1) -> +1 -> DVE recip.
                ex = wpool.tile([128, E], f32, tag="ex")
                nc.scalar.activation(ex[:], ps[:], mybir.ActivationFunctionType.Exp, scale=-1.0)
                u = wpool.tile([128, E], f32, tag="u")
                nc.vector.tensor_scalar(u[:], ex[:], 1.0, None, op0=Alu.add)
                g = wpool.tile([128, E], f32, tag="g")
                nc.vector.reciprocal(g[:], u[:])
                s = wpool.tile([128, E], f32, tag="s")
                nc.vector.tensor_add(s[:], g[:], bias_sb[:])

                # grouped top-2 sums -> group scores [128, 8]
                s3 = s[:].rearrange("p (g e) -> p g e", g=N_GROUPS)
                m1 = spool.tile([128, N_GROUPS], f32, tag="m1")
                nc.vector.tensor_reduce(m1[:], s3, axis=X, op=Alu.max)
                s2 = wpool.tile([128, E], f32, tag="s2")
                nc.vector.match_replace(
                    out=s2[:], in_to_replace=m1[:], in_values=s[:], imm_value=-1e30
                )
                m2 = spool.tile([128, N_GROUPS], f32, tag="m2")
                nc.vector.tensor_reduce(
                    m2[:], s2[:].rearrange("p (g e) -> p g e", g=N_GROUPS), axis=X, op=Alu.max
                )
                gs = spool.tile([128, N_GROUPS], f32, tag="gs")
                nc.vector.tensor_add(gs[:], m1[:], m2[:])

                # top-4 groups: threshold = 4th largest group score
                g8 = spool.tile([128, 8], f32, tag="g8")
                nc.vector.max(g8[:], gs[:])
                gmask = spool.tile([128, N_GROUPS], f32, tag="gmask")
                nc.vector.tensor_scalar(
                    gmask[:], gs[:], g8[:, TOPK_GROUPS - 1 : TOPK_GROUPS], None, op0=Alu.is_ge
                )

                # s_sel = s * gmask (zeros outside selected groups)
                s_sel = wpool.tile([128, E], f32, tag="ssel")
                nc.vector.tensor_tensor(
                    s_sel[:].rearrange("p (g e) -> p g e", g=N_GROUPS),
                    s3,
                    gmask[:].to_broadcast([128, N_GROUPS, EPG]),
                    op=Alu.mult,
                )

                # top-8 experts by biased score
                top8 = spool.tile([128, 8], f32, tag="top8")
                nc.vector.max(top8[:], s_sel[:])
                idx = spool.tile([128, 8], u32, tag="idx")
                nc.vector.max_index(idx[:], top8[:], s_sel[:])

                # positions of the top-8 -> gather sigmoid values (unbiased)
                m8 = wpool.tile([128, E], f32, tag="m8")
                nc.vector.tensor_scalar(m8[:], s_sel[:], top8[:, 7:8], None, op0=Alu.is_ge)
                z = wpool.tile([128, E], f32, tag="z")
                nc.vector.tensor_mul(z[:], g[:], m8[:])
                z8 = spool.tile([128, 8], f32, tag="z8")
                nc.vector.max(z8[:], z[:])
                zidx = spool.tile([128, 8], u32, tag="zidx")
                nc.vector.max_index(zidx[:], z8[:], z[:])

                # align sigmoid values to the biased-score rank order:
                # w8[p, j] = sum_k (idx[p,j] == zidx[p,k]) * z8[p,k]
                idxf = spool.tile([128, 8], f32, tag="idxf")
                nc.vector.tensor_copy(idxf[:], idx[:])
                zidxf = spool.tile([128, 8], f32, tag="zidxf")
                nc.vector.tensor_copy(zidxf[:], zidx[:])
                eq = spool.tile([128, 8, 8], f32, tag="eq")
                nc.vector.tensor_tensor(
                    eq[:],
                    idxf[:].unsqueeze(2).broadcast_to([128, 8, 8]),
                    zidxf[:].unsqueeze(1).broadcast_to([128, 8, 8]),
                    op=Alu.is_equal,
                )
                wm = spool.tile([128, 8, 8], f32, tag="wm")
                nc.vector.tensor_tensor(
                    wm[:], eq[:], z8[:].unsqueeze(1).broadcast_to([128, 8, 8]), op=Alu.mult
                )
                w8 = spool.tile([128, 8], f32, tag="w8")
                nc.vector.tensor_reduce(w8[:], wm[:], axis=X, op=Alu.add)

                # normalize: out = w8 * (2.5 / (sum(w8) + 1e-20))
                den = spool.tile([128, 1], f32, tag="den")
                nc.vector.tensor_reduce(den[:], w8[:], axis=X, op=Alu.add)
                nc.vector.tensor_scalar(
                    den[:], den[:], 1e-20, 1.0 / SCALE, op0=Alu.add, op1=Alu.mult
                )
                rec = spool.tile([128, 1], f32, tag="rec")
                nc.vector.reciprocal(rec[:], den[:])
                wout = spool.tile([128, 8], f32, tag="wout")
                nc.vector.tensor_scalar(wout[:], w8[:], rec[:], None, op0=Alu.mult)

                nc.sync.dma_start(ow_d[tt], wout[:])
                nc.sync.dma_start(oi_d[tt], idx[:])

    nc.finalize()
    return nc


def _get_nc():
    if "nc" not in _CACHE:
        _CACHE["nc"] = _build_nc()
    return _CACHE["nc"]


def _prep_inputs(x_TD, kernel_DE, bias_E):
    # w layout: w_sb[p, k, e] = kernel_DE[k*128 + p, e]
    w_l = np.ascontiguousarray(
        kernel_DE.reshape(KC, 128, E).transpose(1, 0, 2)
    )
    bias_rep = np.ascontiguousarray(np.tile(bias_E[None, :], (128, 1)))
    in_maps = []
    for c in range(N_CORES):
        xc = x_TD[c * TPC : (c + 1) * TPC]  # [1024, 7168]
        # x_sb[p, tt, k, t] = xc[tt*128 + t, k*128 + p]
        xl = np.ascontiguousarray(
            xc.reshape(N_TILES, 128, KC, 128).transpose(3, 0, 2, 1)
        )
        in_maps.append({"x": xl, "w": w_l, "bias": bias_rep})
    return in_maps


def kernel(x_TD, kernel_DE, bias_E, _trace=False):
    from concourse import bass_utils

    x_TD = np.asarray(x_TD, dtype=np.float32)
    kernel_DE = np.asarray(kernel_DE, dtype=np.float32)
    bias_E = np.asarray(bias_E, dtype=np.float32)

    nc = _get_nc()
    in_maps = _prep_inputs(x_TD, kernel_DE, bias_E)
    res = bass_utils.run_bass_kernel_spmd(
        nc, in_maps, core_ids=list(range(N_CORES)), trace=_trace
    )
    _CACHE["last_results"] = res
    weights = np.concatenate(
        [res.results[c]["out_w"].reshape(TPC, TOP_K) for c in range(N_CORES)], axis=0
    )
    indices = np.concatenate(
        [
            res.results[c]["out_idx"].reshape(TPC, TOP_K).astype(np.int32)
            for c in range(N_CORES)
        ],
        axis=0,
    )
    return weights, indices


if __name__ == "__main__":
    rng = np.random.default_rng(0)
    x = rng.standard_normal((T, D), dtype=np.float32)
    w = rng.standard_normal((D, E), dtype=np.float32) / np.sqrt(D)
    b = (rng.standard_normal(E) * 0.01).astype(np.float32)
    wts, idx = kernel(x, w, b)
    print("weights", wts.shape, wts.dtype, "indices", idx.shape, idx.dtype)
    print(wts[:2])
    print(idx[:2])


# revision 5
# speedup vs baseline: 1.0041x; 1.0041x over previous
"""DeepSeekV3 MoE router on 8 TRN2 NeuronCores (Bass/Tile).

Strategy (hardcoded for T=8192, D=7168, E=256, top-k=8, 8 groups, top-4 groups):
  - Data-parallel over tokens: each of 8 cores handles 1024 tokens.
  - Router weight kernel_DE and bias replicated to every core.
  - Host pre-arranges x into the lhsT chunk layout the PE needs
    (contraction dim D on partitions), so no on-chip transposes.
  - Per 128-token tile: 56 accumulating fp32 matmuls -> PSUM scores,
    sigmoid on ACT, grouped top-2 / top-4-groups / top-8 with DVE
    Max8 / max_index / match_replace ops, normalize, DMA out.
"""

import sys

for p in ("/opt/trn_rl_repo", "/root/.axon_site/_ro/trn_rl_repo"):
    if p not in sys.path:
        sys.path.insert(0, p)

import numpy as np

T = 8192
D = 7168
E = 256
TOP_K = 8
N_GROUPS = 8
EPG = E // N_GROUPS  # experts per group = 32
TOPK_GROUPS = 4
SCALE = 2.5
N_CORES = 8
TPC = T // N_CORES  # tokens per core = 1024
N_TILES = TPC // 128  # 8 token tiles per core
KC = D // 128  # 56 contraction chunks

_CACHE = {}


def _build_nc():
    import concourse.bacc as bacc
    import concourse.mybir as mybir
    import concourse.tile as tile

    f32 = mybir.dt.float32
    u32 = mybir.dt.uint32
    X = mybir.AxisListType.X
    Alu = mybir.AluOpType

    nc = bacc.Bacc(trn_type="TRN2")
    x_d = nc.declare_dram_parameter("x", [128, N_TILES, KC, 128], f32, isOutput=False)
    w_d = nc.declare_dram_parameter("w", [128, KC, E], f32, isOutput=False)
    b_d = nc.declare_dram_parameter("bias", [128, E], f32, isOutput=False)
    ow_d = nc.declare_dram_parameter("out_w", [N_TILES, 128, TOP_K], f32, isOutput=True)
    oi_d = nc.declare_dram_parameter("out_idx", [N_TILES, 128, TOP_K], u32, isOutput=True)

    with tile.TileContext(nc) as tc:
        with (
            tc.tile_pool(name="const", bufs=1) as cpool,
            tc.tile_pool(name="xin", bufs=3) as xpool,
            tc.tile_pool(name="work", bufs=2) as wpool,
            tc.tile_pool(name="small", bufs=2) as spool,
            tc.tile_pool(name="psum", bufs=2, space="PSUM") as ppool,
        ):
            # Split the big W load into chunk-groups so tile 0's matmuls can
            # start as soon as the first chunks land instead of stalling
            # ~25us on the full 7.3MB transfer.
            WG = 7  # chunks per W DMA group -> 8 DMAs of ~0.92MB
            w_sb = cpool.tile([128, KC, E], f32)
            for g in range(0, KC, WG):
                nc.sync.dma_start(w_sb[:, g : g + WG], w_d[:, g : g + WG])
            bias_sb = cpool.tile([128, E], f32)
            nc.sync.dma_start(bias_sb[:], b_d[:])

            XG = 14  # chunks per x DMA piece -> 4 DMAs of ~0.92MB per tile
            for tt in range(N_TILES):
                x_sb = xpool.tile([128, KC, 128], f32, tag="x")
                for g in range(0, KC, XG):
                    nc.sync.dma_start(x_sb[:, g : g + XG], x_d[:, tt, g : g + XG])

                ps = ppool.tile([128, E], f32, tag="ps")
                for k in range(KC):
                    nc.tensor.matmul(
                        ps[:],
                        lhsT=x_sb[:, k],
                        rhs=w_sb[:, k],
                        start=(k == 0),
                        stop=(k == KC - 1),
                    )

                # g = sigmoid(scores) = 1/(1+exp(-z)), decomposed exactly as
                # XLA lowers logistic on this backend (bitwise-matching the
                # reference selection): ACT Exp(scale=-1) -> +1 -> DVE recip.
                ex = wpool.tile([128, E], f32, tag="ex")
                nc.scalar.activation(ex[:], ps[:], mybir.ActivationFunctionType.Exp, scale=-1.0)
                u = wpool.tile([128, E], f32, tag="u")
                nc.vector.tensor_scalar(u[:], ex[:], 1.0, None, op0=Alu.add)
                g = wpool.tile([128, E], f32, tag="g")
                nc.vector.reciprocal(g[:], u[:])
                s = wpool.tile([128, E], f32, tag="s")
                nc.vector.tensor_add(s[:], g[:], bias_sb[:])

                # grouped top-2 sums -> group scores [128, 8]
                s3 = s[:].rearrange("p (g e) -> p g e", g=N_GROUPS)
                m1 = spool.tile([128, N_GROUPS], f32, tag="m1")
                nc.vector.tensor_reduce(m1[:], s3, axis=X, op=Alu.max)
                s2 = wpool.tile([128, E], f32, tag="s2")
                nc.vector.match_replace(
                    out=s2[:], in_to_replace=m1[:], in_values=s[:], imm_value=-1e30
                )
                m2 = spool.tile([128, N_GROUPS], f32, tag="m2")
                nc.vector.tensor_reduce(
                    m2[:], s2[:].rearrange("p (g e) -> p g e", g=N_GROUPS), axis=X, op=Alu.max
                )
                gs = spool.tile([128, N_GROUPS], f32, tag="gs")
                nc.vector.tensor_add(gs[:], m1[:], m2[:])

                # top-4 groups: threshold = 4th largest group score
                g8 = spool.tile([128, 8], f32, tag="g8")
                nc.vector.max(g8[:], gs[:])
                gmask = spool.tile([128, N_GROUPS], f32, tag="gmask")
                nc.vector.tensor_scalar(
                    gmask[:], gs[:], g8[:, TOPK_GROUPS - 1 : TOPK_GROUPS], None, op0=Alu.is_ge
                )

                # s_sel = s * gmask (zeros outside selected groups)
                s_sel = wpool.tile([128, E], f32, tag="ssel")
                nc.vector.tensor_tensor(
                    s_sel[:].rearrange("p (g e) -> p g e", g=N_GROUPS),
                    s3,
                    gmask[:].to_broadcast([128, N_GROUPS, EPG]),
                    op=Alu.mult,
                )

                # top-8 experts by biased score
                top8 = spool.tile([128, 8], f32, tag="top8")
                nc.vector.max(top8[:], s_sel[:])
                idx = spool.tile([128, 8], u32, tag="idx")
                nc.vector.max_index(idx[:], top8[:], s_sel[:])

                # positions of the top-8 -> gather sigmoid values (unbiased)
                m8 = wpool.tile([128, E], f32, tag="m8")
                nc.vector.tensor_scalar(m8[:], s_sel[:], top8[:, 7:8], None, op0=Alu.is_ge)
                z = wpool.tile([128, E], f32, tag="z")
                nc.vector.tensor_mul(z[:], g[:], m8[:])
                z8 = spool.tile([128, 8], f32, tag="z8")
                nc.vector.max(z8[:], z[:])
                zidx = spool.tile([128, 8], u32, tag="zidx")
                nc.vector.max_index(zidx[:], z8[:], z[:])

                # align sigmoid values to the biased-score rank order:
                # w8[p, j] = sum_k (idx[p,j] == zidx[p,k]) * z8[p,k]
                idxf = spool.tile([128, 8], f32, tag="idxf")
                nc.vector.tensor_copy(idxf[:], idx[:])
                zidxf = spool.tile([128, 8], f32, tag="zidxf")
                nc.vector.tensor_copy(zidxf[:], zidx[:])
                eq = spool.tile([128, 8, 8], f32, tag="eq")
                nc.vector.tensor_tensor(
                    eq[:],
                    idxf[:].unsqueeze(2).broadcast_to([128, 8, 8]),
                    zidxf[:].unsqueeze(1).broadcast_to([128, 8, 8]),
                    op=Alu.is_equal,
                )
                wm = spool.tile([128, 8, 8], f32, tag="wm")
                nc.vector.tensor_tensor(
                    wm[:], eq[:], z8[:].unsqueeze(1).broadcast_to([128, 8, 8]), op=Alu.mult
                )
                w8 = spool.tile([128, 8], f32, tag="w8")
                nc.vector.tensor_reduce(w8[:], wm[:], axis=X, op=Alu.add)

                # normalize: out = w8 * (2.5 / (sum(w8) + 1e-20))
                den = spool.tile([128, 1], f32, tag="den")
                nc.vector.tensor_reduce(den[:], w8[:], axis=X, op=Alu.add)
                nc.vector.tensor_scalar(
                    den[:], den[:], 1e-20, 1.0 / SCALE, op0=Alu.add, op1=Alu.mult
                )
                rec = spool.tile([128, 1], f32, tag="rec")
                nc.vector.reciprocal(rec[:], den[:])
                wout = spool.tile([128, 8], f32, tag="wout")
                nc.vector.tensor_scalar(wout[:], w8[:], rec[:], None, op0=Alu.mult)

                nc.sync.dma_start(ow_d[tt], wout[:])
                nc.sync.dma_start(oi_d[tt], idx[:])

    nc.finalize()
    return nc


def _get_nc():
    if "nc" not in _CACHE:
        _CACHE["nc"] = _build_nc()
    return _CACHE["nc"]


def _prep_inputs(x_TD, kernel_DE, bias_E):
    # w layout: w_sb[p, k, e] = kernel_DE[k*128 + p, e]
    w_l = np.ascontiguousarray(
        kernel_DE.reshape(KC, 128, E).transpose(1, 0, 2)
    )
    bias_rep = np.ascontiguousarray(np.tile(bias_E[None, :], (128, 1)))
    in_maps = []
    for c in range(N_CORES):
        xc = x_TD[c * TPC : (c + 1) * TPC]  # [1024, 7168]
        # x_sb[p, tt, k, t] = xc[tt*128 + t, k*128 + p]
        xl = np.ascontiguousarray(
            xc.reshape(N_TILES, 128, KC, 128).transpose(3, 0, 2, 1)
        )
        in_maps.append({"x": xl, "w": w_l, "bias": bias_rep})
    return in_maps


def kernel(x_TD, kernel_DE, bias_E, _trace=False):
    from concourse import bass_utils

    x_TD = np.asarray(x_TD, dtype=np.float32)
    kernel_DE = np.asarray(kernel_DE, dtype=np.float32)
    bias_E = np.asarray(bias_E, dtype=np.float32)

    nc = _get_nc()
    in_maps = _prep_inputs(x_TD, kernel_DE, bias_E)
    res = bass_utils.run_bass_kernel_spmd(
        nc, in_maps, core_ids=list(range(N_CORES)), trace=_trace
    )
    _CACHE["last_results"] = res
    weights = np.concatenate(
        [res.results[c]["out_w"].reshape(TPC, TOP_K) for c in range(N_CORES)], axis=0
    )
    indices = np.concatenate(
        [
            res.results[c]["out_idx"].reshape(TPC, TOP_K).astype(np.int32)
            for c in range(N_CORES)
        ],
        axis=0,
    )
    return weights, indices


if __name__ == "__main__":
    rng = np.random.default_rng(0)
    x = rng.standard_normal((T, D), dtype=np.float32)
    w = rng.standard_normal((D, E), dtype=np.float32) / np.sqrt(D)
    b = (rng.standard_normal(E) * 0.01).astype(np.float32)
    wts, idx = kernel(x, w, b)
    print("weights", wts.shape, wts.dtype, "indices", idx.shape, idx.dtype)
    print(wts[:2])
    print(idx[:2])


# revision 7
# speedup vs baseline: 1.0271x; 1.0229x over previous
"""DeepSeekV3 MoE router on 8 TRN2 NeuronCores (Bass/Tile).

Strategy (hardcoded for T=8192, D=7168, E=256, top-k=8, 8 groups, top-4 groups):
  - Data-parallel over tokens: each of 8 cores handles 1024 tokens.
  - Router weight kernel_DE and bias replicated to every core.
  - Host pre-arranges x into the lhsT chunk layout the PE needs
    (contraction dim D on partitions), so no on-chip transposes.
  - Per 128-token tile: 56 accumulating fp32 matmuls -> PSUM scores,
    sigmoid on ACT, grouped top-2 / top-4-groups / top-8 with DVE
    Max8 / max_index / match_replace ops, normalize, DMA out.
"""

import sys

for p in ("/opt/trn_rl_repo", "/root/.axon_site/_ro/trn_rl_repo"):
    if p not in sys.path:
        sys.path.insert(0, p)

import numpy as np

T = 8192
D = 7168
E = 256
TOP_K = 8
N_GROUPS = 8
EPG = E // N_GROUPS  # experts per group = 32
TOPK_GROUPS = 4
SCALE = 2.5
N_CORES = 8
TPC = T // N_CORES  # tokens per core = 1024
N_TILES = TPC // 128  # 8 token tiles per core
KC = D // 128  # 56 contraction chunks

_CACHE = {}


def _build_nc():
    import concourse.bacc as bacc
    import concourse.mybir as mybir
    import concourse.tile as tile

    f32 = mybir.dt.float32
    u32 = mybir.dt.uint32
    X = mybir.AxisListType.X
    Alu = mybir.AluOpType

    nc = bacc.Bacc(trn_type="TRN2")
    x_d = nc.declare_dram_parameter("x", [128, N_TILES, KC, 128], f32, isOutput=False)
    w_d = nc.declare_dram_parameter("w", [128, KC, E], f32, isOutput=False)
    b_d = nc.declare_dram_parameter("bias", [128, E], f32, isOutput=False)
    ow_d = nc.declare_dram_parameter("out_w", [N_TILES, 128, TOP_K], f32, isOutput=True)
    oi_d = nc.declare_dram_parameter("out_idx", [N_TILES, 128, TOP_K], u32, isOutput=True)

    with tile.TileContext(nc) as tc:
        with (
            tc.tile_pool(name="const", bufs=1) as cpool,
            tc.tile_pool(name="xin", bufs=3) as xpool,
            tc.tile_pool(name="work", bufs=2) as wpool,
            tc.tile_pool(name="small", bufs=2) as spool,
            tc.tile_pool(name="psum", bufs=2, space="PSUM") as ppool,
        ):
            # W and x are loaded as SEPARATE tiles per chunk-group so Tile's
            # per-tile dependency tracking lets tile 0's first matmuls start
            # as soon as the first groups land (instead of stalling on the
            # whole 7.3MB W + 3.7MB x transfer).
            WG = 7  # chunks per W group -> 8 tiles of ~0.92MB
            XG = 14  # chunks per x piece -> 4 tiles of ~0.92MB per token tile
            bias_sb = cpool.tile([128, E], f32)
            nc.sync.dma_start(bias_sb[:], b_d[:])
            w_gs = []
            for g in range(KC // WG):
                w_g = cpool.tile([128, WG, E], f32, tag=f"w{g}")
                nc.sync.dma_start(w_g[:], w_d[:, g * WG : (g + 1) * WG])
                w_gs.append(w_g)

            for tt in range(N_TILES):
                x_ps = []
                for g in range(KC // XG):
                    x_p = xpool.tile([128, XG, 128], f32, tag=f"x{g}")
                    nc.sync.dma_start(x_p[:], x_d[:, tt, g * XG : (g + 1) * XG])
                    x_ps.append(x_p)

                ps = ppool.tile([128, E], f32, tag="ps")
                for k in range(KC):
                    nc.tensor.matmul(
                        ps[:],
                        lhsT=x_ps[k // XG][:, k % XG],
                        rhs=w_gs[k // WG][:, k % WG],
                        start=(k == 0),
                        stop=(k == KC - 1),
                    )

                # g = sigmoid(scores) = 1/(1+exp(-z)), decomposed exactly as
                # XLA lowers logistic on this backend (bitwise-matching the
                # reference selection): ACT Exp(scale=-1) -> +1 -> DVE recip.
                ex = wpool.tile([128, E], f32, tag="ex")
                nc.scalar.activation(ex[:], ps[:], mybir.ActivationFunctionType.Exp, scale=-1.0)
                u = wpool.tile([128, E], f32, tag="u")
                nc.vector.tensor_scalar(u[:], ex[:], 1.0, None, op0=Alu.add)
                g = wpool.tile([128, E], f32, tag="g")
                nc.vector.reciprocal(g[:], u[:])
                s = wpool.tile([128, E], f32, tag="s")
                nc.vector.tensor_add(s[:], g[:], bias_sb[:])

                # grouped top-2 sums -> group scores [128, 8]
                s3 = s[:].rearrange("p (g e) -> p g e", g=N_GROUPS)
                m1 = spool.tile([128, N_GROUPS], f32, tag="m1")
                nc.vector.tensor_reduce(m1[:], s3, axis=X, op=Alu.max)
                s2 = wpool.tile([128, E], f32, tag="s2")
                nc.vector.match_replace(
                    out=s2[:], in_to_replace=m1[:], in_values=s[:], imm_value=-1e30
                )
                m2 = spool.tile([128, N_GROUPS], f32, tag="m2")
                nc.vector.tensor_reduce(
                    m2[:], s2[:].rearrange("p (g e) -> p g e", g=N_GROUPS), axis=X, op=Alu.max
                )
                gs = spool.tile([128, N_GROUPS], f32, tag="gs")
                nc.vector.tensor_add(gs[:], m1[:], m2[:])

                # top-4 groups: threshold = 4th largest group score
                g8 = spool.tile([128, 8], f32, tag="g8")
                nc.vector.max(g8[:], gs[:])
                gmask = spool.tile([128, N_GROUPS], f32, tag="gmask")
                nc.vector.tensor_scalar(
                    gmask[:], gs[:], g8[:, TOPK_GROUPS - 1 : TOPK_GROUPS], None, op0=Alu.is_ge
                )

                # s_sel = s * gmask (zeros outside selected groups)
                s_sel = wpool.tile([128, E], f32, tag="ssel")
                nc.vector.tensor_tensor(
                    s_sel[:].rearrange("p (g e) -> p g e", g=N_GROUPS),
                    s3,
                    gmask[:].to_broadcast([128, N_GROUPS, EPG]),
                    op=Alu.mult,
                )

                # top-8 experts by biased score
                top8 = spool.tile([128, 8], f32, tag="top8")
                nc.vector.max(top8[:], s_sel[:])
                idx = spool.tile([128, 8], u32, tag="idx")
                nc.vector.max_index(idx[:], top8[:], s_sel[:])

                # positions of the top-8 -> gather sigmoid values (unbiased)
                m8 = wpool.tile([128, E], f32, tag="m8")
                nc.vector.tensor_scalar(m8[:], s_sel[:], top8[:, 7:8], None, op0=Alu.is_ge)
                z = wpool.tile([128, E], f32, tag="z")
                nc.vector.tensor_mul(z[:], g[:], m8[:])
                z8 = spool.tile([128, 8], f32, tag="z8")
                nc.vector.max(z8[:], z[:])
                zidx = spool.tile([128, 8], u32, tag="zidx")
                nc.vector.max_index(zidx[:], z8[:], z[:])

                # align sigmoid values to the biased-score rank order:
                # w8[p, j] = sum_k (idx[p,j] == zidx[p,k]) * z8[p,k]
                idxf = spool.tile([128, 8], f32, tag="idxf")
                nc.vector.tensor_copy(idxf[:], idx[:])
                zidxf = spool.tile([128, 8], f32, tag="zidxf")
                nc.vector.tensor_copy(zidxf[:], zidx[:])
                eq = spool.tile([128, 8, 8], f32, tag="eq")
                nc.vector.tensor_tensor(
                    eq[:],
                    idxf[:].unsqueeze(2).broadcast_to([128, 8, 8]),
                    zidxf[:].unsqueeze(1).broadcast_to([128, 8, 8]),
                    op=Alu.is_equal,
                )
                wm = spool.tile([128, 8, 8], f32, tag="wm")
                nc.vector.tensor_tensor(
                    wm[:], eq[:], z8[:].unsqueeze(1).broadcast_to([128, 8, 8]), op=Alu.mult
                )
                w8 = spool.tile([128, 8], f32, tag="w8")
                nc.vector.tensor_reduce(w8[:], wm[:], axis=X, op=Alu.add)

                # normalize: out = w8 * (2.5 / (sum(w8) + 1e-20))
                den = spool.tile([128, 1], f32, tag="den")
                nc.vector.tensor_reduce(den[:], w8[:], axis=X, op=Alu.add)
                nc.vector.tensor_scalar(
                    den[:], den[:], 1e-20, 1.0 / SCALE, op0=Alu.add, op1=Alu.mult
                )
                rec = spool.tile([128, 1], f32, tag="rec")
                nc.vector.reciprocal(rec[:], den[:])
                wout = spool.tile([128, 8], f32, tag="wout")
                nc.vector.tensor_scalar(wout[:], w8[:], rec[:], None, op0=Alu.mult)

                nc.sync.dma_start(ow_d[tt], wout[:])
                nc.sync.dma_start(oi_d[tt], idx[:])

    nc.finalize()
    return nc


def _get_nc():
    if "nc" not in _CACHE:
        _CACHE["nc"] = _build_nc()
    return _CACHE["nc"]


def _prep_inputs(x_TD, kernel_DE, bias_E):
    # w layout: w_sb[p, k, e] = kernel_DE[k*128 + p, e]
    w_l = np.ascontiguousarray(
        kernel_DE.reshape(KC, 128, E).transpose(1, 0, 2)
    )
    bias_rep = np.ascontiguousarray(np.tile(bias_E[None, :], (128, 1)))
    in_maps = []
    for c in range(N_CORES):
        xc = x_TD[c * TPC : (c + 1) * TPC]  # [1024, 7168]
        # x_sb[p, tt, k, t] = xc[tt*128 + t, k*128 + p]
        xl = np.ascontiguousarray(
            xc.reshape(N_TILES, 128, KC, 128).transpose(3, 0, 2, 1)
        )
        in_maps.append({"x": xl, "w": w_l, "bias": bias_rep})
    return in_maps


def kernel(x_TD, kernel_DE, bias_E, _trace=False):
    from concourse import bass_utils

    x_TD = np.asarray(x_TD, dtype=np.float32)
    kernel_DE = np.asarray(kernel_DE, dtype=np.float32)
    bias_E = np.asarray(bias_E, dtype=np.float32)

    nc = _get_nc()
    in_maps = _prep_inputs(x_TD, kernel_DE, bias_E)
    res = bass_utils.run_bass_kernel_spmd(
        nc, in_maps, core_ids=list(range(N_CORES)), trace=_trace
    )
    _CACHE["last_results"] = res
    weights = np.concatenate(
        [res.results[c]["out_w"].reshape(TPC, TOP_K) for c in range(N_CORES)], axis=0
    )
    indices = np.concatenate(
        [
            res.results[c]["out_idx"].reshape(TPC, TOP_K).astype(np.int32)
            for c in range(N_CORES)
        ],
        axis=0,
    )
    return weights, indices


if __name__ == "__main__":
    rng = np.random.default_rng(0)
    x = rng.standard_normal((T, D), dtype=np.float32)
    w = rng.standard_normal((D, E), dtype=np.float32) / np.sqrt(D)
    b = (rng.standard_normal(E) * 0.01).astype(np.float32)
    wts, idx = kernel(x, w, b)
    print("weights", wts.shape, wts.dtype, "indices", idx.shape, idx.dtype)
    print(wts[:2])
    print(idx[:2])
